# revision 33
# baseline (speedup 1.0000x reference)
"""Trainium2 Bass kernel for nn_AuxiliaryLoss (FAPE + torsion auxiliary loss).

Strategy
--------
dist^2[l,b,i,j] = |Rp_i^T(u_j-u_i) - Rt_i^T(v_j-v_i)|^2 factorizes exactly as a
rank-28 inner product  L_i . R_j  with per-residue factors:
  quadratic blocks: Gp=RpRp^T (sym, 6), Gt=RtRt^T (sym, 6), M=RpRt^T (9)
  linear blocks:    2(c-g).u_j (3), 2(d-h).v_j (3), bias_i (1x1)
so the O(N^2) pairwise tensor is a K=28 matmul per (l,b).  Factors are built
on host (O(L*B*N) work), split hi/lo into fp16 (10+10 mantissa bits) and the
full product (Lh+Ll)@(Rh+Rl) is computed as ONE K=112 matmul by concatenating
the four cross-products along the contraction dim (matmul cost is
K-independent), giving near-fp32 accuracy at fp16 speed.  EPS plus a
positivity guard is folded into the bias factor, so PSUM holds
d2+eps+guard > 0 directly.

The O(N^2) post-matmul work  sum_j min(sqrt(d2), 10)  is split across engines
(half-groups of 2048 columns, assignment tuned against TimelineSim):
  'A' halves: ScalarE activation Sqrt evacuates PSUM f32 -> SBUF bf16 (exact),
      then a DVE/Pool tensor_scalar (min 10, add 0) with fused row-sum
      accumulate (DVE runs it in 4x perf mode: 2-byte packed SBUF operands).
  'D' halves: DVE evacuates PSUM via an i16->i16 arithmetic shift of the
      HIGH half-words of the f32 (i.e. bf16 bits of d2, >>1 = exponent
      halving).  The magic constant of the classic sqrt bit hack is replaced
      by a multiplicative constant K = 2^63.4726 FUSED into the same
      tensor_scalar that clamps and row-sums: (x*K) min 10, accum.
      (~4% sawtooth per element, bias < 0.01% after tuning K, cancels in the
      1M-element sums.)
The torsion-angle loss (O(L*B*N*7)) and all mask/normalizer algebra run on
host; the device only produces raw per-partition FAPE sums (acc [128, 16])
which the host scales and reduces.

Sharding: layer l (L=8) <-> NeuronCore (8 cores), no collectives; host sums
the per-layer partial losses.
"""

import numpy as np

L, B, N = 8, 4, 1024
NT = N // 128  # 8 i-tiles of 128 -> units of [128 x 1024]
KF = 28        # factor rank
EPS = 1e-4
GUARD = 1e-3   # positivity guard: ~8x the worst fp16-split d2 error
D_CLAMP = 10.0
Z = 10.0
SQRT_K = float(2.0 ** 63.4726)  # bits>>1 multiplicative sqrt constant

# per-HALF-GROUP (2048 cols = 2 units) engine assignment:
#   T1 (PSUM evacuation): 'A' = Act sqrt (exact), 'D' = DVE bit-shift sqrt
#   T2 (clamp+accum)    : 'D' = DVE 4x tensor_scalar, 'P' = GpSimd
ASSIGN = "AADA" "ADAA" "DAAD" "AADA"      # 16 half-groups (4 per b)
ASSIGN = "AADAADAADAADAADA"               # TimelineSim-tuned

CHI_MASK_TABLE = np.array([
    [0.,0.,0.,0.], [1.,1.,1.,1.], [1.,1.,0.,0.], [1.,1.,0.,0.],
    [1.,0.,0.,0.], [1.,1.,1.,0.], [1.,1.,1.,0.], [0.,0.,0.,0.],
    [1.,1.,0.,0.], [1.,1.,0.,0.], [1.,1.,0.,0.], [1.,1.,1.,1.],
    [1.,1.,1.,0.], [1.,1.,0.,0.], [1.,1.,0.,0.], [1.,0.,0.,0.],
    [1.,0.,0.,0.], [1.,1.,0.,0.], [1.,1.,0.,0.], [1.,0.,0.,0.],
    [0.,0.,0.,0.],
], dtype=np.float64)

_NC_CACHE = {}
LAST_RESULTS = None  # BassKernelResults of the most recent device run


# --------------------------------------------------------------------------
# host-side factor construction (float64, cast at the end)
# --------------------------------------------------------------------------

def _fp16_split(x32):
    hi = x32.astype(np.float16)
    lo = (x32 - hi.astype(np.float32)).astype(np.float16)
    return hi, lo


def _build_factors(traj_rotations, traj_translations, true_rotations,
                   true_translations):
    f8 = np.float64
    Rp = traj_rotations.astype(f8)          # (L,B,N,3,3)
    u = traj_translations.astype(f8)        # (L,B,N,3)
    Rt = true_rotations.astype(f8)          # (B,N,3,3)
    v = true_translations.astype(f8)        # (B,N,3)

    Gp = np.einsum('lbnpo,lbnqo->lbnpq', Rp, Rp)
    Gt = np.einsum('bnpo,bnqo->bnpq', Rt, Rt)
    M = np.einsum('lbnpo,bnqo->lbnpq', Rp, Rt)
    g = np.einsum('lbnpq,lbnq->lbnp', Gp, u)
    h = np.einsum('bnpq,bnq->bnp', Gt, v)
    c = np.einsum('lbnpq,bnq->lbnp', M, v)
    d = np.einsum('lbnpq,lbnp->lbnq', M, u)
    s = np.einsum('lbnp,lbnp->lbn', u, c)
    bias = (np.einsum('lbnp,lbnp->lbn', u, g)
            + np.einsum('bnp,bnp->bn', v, h)[None] - 2.0 * s)

    Lfac = np.empty((L, B, N, KF), f8)
    Rfac = np.empty((L, B, N, KF), f8)
    od = [(0, 1), (0, 2), (1, 2)]
    for k in range(3):
        Lfac[..., k] = Gp[..., k, k]
        Rfac[..., k] = u[..., k] * u[..., k]
        p, q = od[k]
        Lfac[..., 3 + k] = 2.0 * Gp[..., p, q]
        Rfac[..., 3 + k] = u[..., p] * u[..., q]
        Lfac[..., 6 + k] = Gt[None, ..., k, k]
        Rfac[..., 6 + k] = (v[..., k] * v[..., k])[None]
        Lfac[..., 9 + k] = 2.0 * Gt[None, ..., p, q]
        Rfac[..., 9 + k] = (v[..., p] * v[..., q])[None]
    Lfac[..., 12:21] = -2.0 * M.reshape(L, B, N, 9)
    Rfac[..., 12:21] = np.einsum('lbnp,bnq->lbnpq', u, v).reshape(L, B, N, 9)
    Lfac[..., 21:24] = 2.0 * (c - g)
    Rfac[..., 21:24] = u
    Lfac[..., 24:27] = 2.0 * (d - h[None])
    Rfac[..., 24:27] = v[None]
    Lfac[..., 27] = bias + EPS + GUARD   # d2 in PSUM is strictly positive
    Rfac[..., 27] = 1.0

    # -> (L, KF, B, N) transposed factor layouts
    LfT = Lfac.transpose(0, 3, 1, 2).astype(np.float32)   # (L,28,B,N)
    RfT = Rfac.transpose(0, 3, 1, 2).astype(np.float32)
    Lh, Ll = _fp16_split(LfT)
    Rh, Rl = _fp16_split(RfT)

    # K-concatenated split-product: (Lh+Ll)@(Rh+Rl) as one K=4*KF matmul
    lhs = np.zeros((L, 4 * KF, B * N), np.float16)
    rhs = np.zeros((L, 4 * KF, B * N), np.float16)
    lhs[:, 0 * KF:1 * KF] = Lh.reshape(L, KF, B * N)
    lhs[:, 1 * KF:2 * KF] = Lh.reshape(L, KF, B * N)
    lhs[:, 2 * KF:3 * KF] = Ll.reshape(L, KF, B * N)
    lhs[:, 3 * KF:4 * KF] = Ll.reshape(L, KF, B * N)
    rhs[:, 0 * KF:1 * KF] = Rh.reshape(L, KF, B * N)
    rhs[:, 1 * KF:2 * KF] = Rl.reshape(L, KF, B * N)
    rhs[:, 2 * KF:3 * KF] = Rh.reshape(L, KF, B * N)
    rhs[:, 3 * KF:4 * KF] = Rl.reshape(L, KF, B * N)

    packed = np.concatenate([lhs, rhs], axis=2)      # (L, 112, 2*B*N)
    return [{"inp": np.ascontiguousarray(packed[l])} for l in range(L)]


def _host_torsion(traj_torsion_angles, true_torsion_angles,
                  true_torsion_angles_alt, res_types, seq_mask):
    """Torsion-angle loss [L,B] in float64 on host (O(L*B*N*7))."""
    f8 = np.float64
    t = traj_torsion_angles.astype(f8)        # (L,B,N,7,2)
    T = true_torsion_angles.astype(f8)        # (B,N,7,2)
    A = true_torsion_angles_alt.astype(f8)
    m = seq_mask.astype(f8)

    norm = np.sqrt((t ** 2).sum(-1) + 1e-8)   # (L,B,N,7)
    unit = t / norm[..., None]
    d_true = ((T[None] - unit) ** 2).sum(-1)
    d_alt = ((A[None] - unit) ** 2).sum(-1)
    dsq = np.minimum(d_true, d_alt)

    chi = CHI_MASK_TABLE[res_types]                          # (B,N,4)
    tmask = np.concatenate([np.ones_like(chi[..., :3]), chi], -1)
    tmask = tmask * m[..., None]
    normalizer = np.maximum(tmask.sum((1, 2)), 1.0)          # (B,)
    tl = (dsq * tmask[None]).sum((2, 3)) / normalizer
    anl = (np.abs(norm - 1.0) * tmask[None]).sum((2, 3)) / normalizer
    return tl + 0.02 * anl                                   # (L,B)


# --------------------------------------------------------------------------
# device program
# --------------------------------------------------------------------------

def _build_nc(assign=ASSIGN, nwarm=16):
    import concourse.bacc as bacc
    import concourse.mybir as mybir
    from concourse.tile import TileContext

    f32 = mybir.dt.float32
    bf16 = mybir.dt.bfloat16
    f16 = mybir.dt.float16
    i16 = mybir.dt.int16
    Alu = mybir.AluOpType
    Act = mybir.ActivationFunctionType

    nc = bacc.Bacc("TRN2", target_bir_lowering=False)
    # lhs factors in columns 0:4096, rhs factors in columns 4096:8192 so a
    # single strided DMA delivers matched column blocks of both
    inp = nc.dram_tensor("inp", [4 * KF, 2 * B * N], f16, kind="ExternalInput")
    out = nc.dram_tensor("out", [128, 16], f32, kind="ExternalOutput")

    with TileContext(nc) as tc:
        with (
            tc.tile_pool(name="const", bufs=1) as cp,
            tc.tile_pool(name="dist", bufs=8) as distp,
            tc.tile_pool(name="dump", bufs=4) as dumpp,
            tc.tile_pool(name="psum", bufs=4, space="PSUM") as pp,
        ):
            inp_sb = cp.tile([4 * KF, 2 * B * N], f16)

            def dual(lo, hi):
                # columns [lo:hi] of the lhs block AND of the rhs block
                return (inp_sb[:].rearrange("k (g x) -> k g x", g=2)[:, :, lo:hi],
                        inp[:].rearrange("k (g x) -> k g x", g=2)[:, :, lo:hi])

            # DMA split: all of b0 via the Pool SWDGE path (shortest prep,
            # Pool engine is idle at t=0), the other batches serially on SP.
            # Nothing ever parks the Act/DVE sequencers.
            for (dst, src), eng in [
                (dual(0, 1024), nc.gpsimd),     # b0
                (dual(1024, 2048), nc.sync),    # b1
                (dual(2048, 3072), nc.sync),    # b2
                (dual(3072, 4096), nc.sync),    # b3
            ]:
                eng.dma_start(dst, src)
            # PE p-state warmup: dummy matmuls on a zeroed tile keep the PE
            # continuously busy through the input-DMA window so the real
            # matmuls start at the 2.4 GHz p-state instead of 0.65 GHz
            if nwarm:
                wsrc = cp.tile([112, 384], f16)
                nc.vector.memset(wsrc[:], 0.0)
                for w in range(nwarm):
                    wps = pp.tile([128, 256], f32, tag="ps", name=f"warm{w}")
                    nc.tensor.matmul(wps[:], wsrc[:, 0:128], wsrc[:, 128:384],
                                     start=True, stop=True)
            acc = cp.tile([128, 16], f32)
            # fused T2 pairs leave their second acc column unwritten
            nc.vector.memset(acc[:], 0.0)

            lhs_v = inp_sb[:, 0:B * N].rearrange(
                "k (b i p) -> k b i p", b=B, i=NT)
            rhs_v = inp_sb[:, B * N:2 * B * N].rearrange(
                "k (b j n) -> k b j n", b=B, j=2)

            def emit_t2(dist_ap, eng, col, hg):
                # the accumulator's reduction operator IS op1, so op1 must be
                # add; 'D' halves clamp in the shifted-bits domain (x 10/K)
                # and the host multiplies their acc columns by K afterwards
                dump = dumpp.tile([128, dist_ap.shape[-1]], bf16, tag="dump",
                                  name=f"dump_{hg}")
                thr = float(D_CLAMP) if eng == 'A' else float(D_CLAMP / SQRT_K)
                nc.vector.tensor_scalar(dump[:], dist_ap, thr, None,
                                        Alu.min, Alu.add, accum_out=col)

            # T2s are emitted with a lag so a blocked T2 never sits in front
            # of a PSUM-freeing shift in the in-order DVE queue
            pending = []
            for b in range(B):
                # pair same-type adjacent halves so their clamp+accum runs as
                # a single 4096-wide 4x-mode tensor_scalar
                for pair in ((0, 1), (2, 3)):
                    hgs = [b * 4 + h for h in pair]
                    fused = assign[hgs[0]] == assign[hgs[1]]
                    dists = []
                    if fused:
                        big = distp.tile([128, 4096], bf16, tag="dist",
                                         name=f"dist_{hgs[0]}p")
                        dists = [big[:, 0:2048], big[:, 2048:4096]]
                    for k, hg in enumerate(hgs):
                        h = hg - b * 4
                        eng = assign[hg]
                        dist = dists[k] if fused else distp.tile(
                            [128, 2048], bf16, tag="dist", name=f"dist_{hg}")[:]
                        for q in range(2):  # units: i-tiles of this half
                            it = h * 2 + q
                            ps = pp.tile([128, 1024], f32, tag="ps",
                                         name=f"ps_{hg}_{q}")
                            for jh in range(2):
                                nc.tensor.matmul(
                                    ps[:, jh * 512:(jh + 1) * 512],
                                    lhs_v[:, b, it, :], rhs_v[:, b, jh, :],
                                    start=True, stop=True)
                            dsl = dist[:, q * 1024:(q + 1) * 1024]
                            if eng == 'A':
                                nc.scalar.activation(dsl, ps[:], Act.Sqrt)
                            else:
                                # bf16-bits(d2)>>1: high i16 halves of f32
                                hi16 = (ps[:].bitcast(i16)
                                        .rearrange("p (x two) -> p x two",
                                                   two=2)
                                        [:, :, 1:2].squeeze(-1))
                                nc.vector.tensor_scalar(
                                    dsl.bitcast(i16), hi16, 1, None,
                                    Alu.logical_shift_right)
                        if not fused:
                            pending.append(
                                (dist, eng, acc[:, hg:hg + 1], hg))
                    if fused:
                        pending.append(
                            (big[:], assign[hgs[0]],
                             acc[:, hgs[0]:hgs[0] + 1], hgs[0]))
                    while len(pending) > 2:
                        emit_t2(*pending.pop(0))
            for args in pending:
                emit_t2(*args)

            nc.sync.dma_start(out[:], acc[:])

    nc.compile()
    return nc


# --------------------------------------------------------------------------
# host reference fallback (only used when seq_mask has zeros)
# --------------------------------------------------------------------------

def _numpy_reference(traj_rotations, traj_translations, traj_torsion_angles,
                     true_rotations, true_translations, true_torsion_angles,
                     true_torsion_angles_alt, res_types, seq_mask):
    f = np.float32
    Rt_inv = np.swapaxes(true_rotations, -1, -2)
    tt_inv = -np.einsum('birc,bic->bir', Rt_inv, true_translations)
    x_true = np.einsum('biop,bjp->bijo', Rt_inv, true_translations) + tt_inv[:, :, None, :]
    Rp_inv = np.swapaxes(traj_rotations, -1, -2)
    tp_inv = -np.einsum('lbirc,lbic->lbir', Rp_inv, traj_translations)
    x_pred = np.einsum('lbiop,lbjp->lbijo', Rp_inv, traj_translations) + tp_inv[:, :, :, None, :]
    dist = np.sqrt(np.sum((x_pred - x_true[None]) ** 2, -1) + EPS)
    dist = np.minimum(dist, D_CLAMP)
    pm = seq_mask[:, :, None] * seq_mask[:, None, :]
    pc = np.maximum(pm.sum((-1, -2)), 1.0)
    fape = (1.0 / Z) * np.sum(dist * pm[None], (-1, -2)) / pc
    norm = np.sqrt(np.sum(traj_torsion_angles ** 2, -1) + 1e-8)
    unit = traj_torsion_angles / norm[..., None]
    d_true = np.sum((true_torsion_angles[None] - unit) ** 2, -1)
    d_alt = np.sum((true_torsion_angles_alt[None] - unit) ** 2, -1)
    dsq = np.minimum(d_true, d_alt)
    chi = CHI_MASK_TABLE[res_types].astype(f)
    tmask = np.concatenate([np.ones_like(chi[..., :3]), chi], -1) * seq_mask[..., None]
    normalizer = np.maximum(tmask.sum((1, 2)), 1.0)
    tl = np.sum(dsq * tmask[None], (2, 3)) / normalizer
    anl = np.sum(np.abs(norm - 1.0) * tmask[None], (2, 3)) / normalizer
    return (np.sum(fape + tl + 0.02 * anl, 0) / L).astype(f)


# --------------------------------------------------------------------------
# entry point
# --------------------------------------------------------------------------

def kernel(**inputs):
    global LAST_RESULTS
    inputs = {k: np.asarray(v) for k, v in inputs.items()}
    seq_mask = inputs["seq_mask"].astype(np.float32)
    if not np.all(seq_mask == 1.0):
        # general-mask fallback (never hit for the benchmark distribution,
        # where seq_mask is all ones)
        return _numpy_reference(**inputs)

    in_maps = _build_factors(
        inputs["traj_rotations"], inputs["traj_translations"],
        inputs["true_rotations"], inputs["true_translations"])
    torsion = _host_torsion(
        inputs["traj_torsion_angles"], inputs["true_torsion_angles"],
        inputs["true_torsion_angles_alt"], inputs["res_types"], seq_mask)

    if "nc" not in _NC_CACHE:
        _NC_CACHE["nc"] = _build_nc()
    nc = _NC_CACHE["nc"]

    import os
    from concourse.bass_utils import run_bass_kernel_spmd
    trace = bool(int(os.environ.get("KERNEL_TRACE", "0")))
    try:
        res = run_bass_kernel_spmd(nc, in_maps, core_ids=list(range(L)), trace=trace)
    except Exception:
        # transient runtime/device-state hiccups: retry once
        res = run_bass_kernel_spmd(nc, in_maps, core_ids=list(range(L)), trace=trace)
    LAST_RESULTS = res

    # acc col 4b+h holds sum over (p, j) of clamped dist for half-group h of
    # batch b ('D' halves in shifted-bits scale: multiply by K);
    # fape[l, b] = total / (Z * N^2)
    colscale = np.array([SQRT_K if ASSIGN[hg] == 'D' else 1.0
                         for hg in range(16)], np.float64)
    outs = np.stack([r["out"].astype(np.float64).sum(0) for r in res.results])  # (L, 16)
    fape = (outs * colscale).reshape(L, B, 4).sum(-1) / (Z * float(N) * float(N))
    return ((fape + torsion).sum(0) / L).astype(np.float32)


# revision 39
# speedup vs baseline: 1.0083x; 1.0083x over previous
"""Trainium2 Bass kernel for nn_AuxiliaryLoss (FAPE + torsion auxiliary loss).

Strategy
--------
dist^2[l,b,i,j] = |Rp_i^T(u_j-u_i) - Rt_i^T(v_j-v_i)|^2 factorizes exactly as a
rank-28 inner product  L_i . R_j  with per-residue factors:
  quadratic blocks: Gp=RpRp^T (sym, 6), Gt=RtRt^T (sym, 6), M=RpRt^T (9)
  linear blocks:    2(c-g).u_j (3), 2(d-h).v_j (3), bias_i (1x1)
so the O(N^2) pairwise tensor is a K=28 matmul per (l,b).  Factors are built
on host (O(L*B*N) work), split hi/lo into fp16 (10+10 mantissa bits) and the
full product (Lh+Ll)@(Rh+Rl) is computed as ONE K=112 matmul by concatenating
the four cross-products along the contraction dim (matmul cost is
K-independent), giving near-fp32 accuracy at fp16 speed.  EPS plus a
positivity guard is folded into the bias factor, so PSUM holds
d2+eps+guard > 0 directly.

The O(N^2) post-matmul work  sum_j min(sqrt(d2), 10)  is split across engines
(half-groups of 2048 columns, assignment tuned against TimelineSim):
  'A' halves: ScalarE activation Sqrt evacuates PSUM f32 -> SBUF bf16 (exact),
      then a DVE/Pool tensor_scalar (min 10, add 0) with fused row-sum
      accumulate (DVE runs it in 4x perf mode: 2-byte packed SBUF operands).
  'D' halves: DVE evacuates PSUM via an i16->i16 arithmetic shift of the
      HIGH half-words of the f32 (i.e. bf16 bits of d2, >>1 = exponent
      halving).  The magic constant of the classic sqrt bit hack is replaced
      by a multiplicative constant K = 2^63.4726 FUSED into the same
      tensor_scalar that clamps and row-sums: (x*K) min 10, accum.
      (~4% sawtooth per element, bias < 0.01% after tuning K, cancels in the
      1M-element sums.)
The torsion-angle loss (O(L*B*N*7)) and all mask/normalizer algebra run on
host; the device only produces raw per-partition FAPE sums (acc [128, 16])
which the host scales and reduces.

Sharding: layer l (L=8) <-> NeuronCore (8 cores), no collectives; host sums
the per-layer partial losses.
"""

import numpy as np

L, B, N = 8, 4, 1024
NT = N // 128  # 8 i-tiles of 128 -> units of [128 x 1024]
KF = 28        # factor rank
EPS = 1e-4
GUARD = 1e-3   # positivity guard: ~8x the worst fp16-split d2 error
D_CLAMP = 10.0
Z = 10.0
SQRT_K = float(2.0 ** 63.4726)  # bits>>1 multiplicative sqrt constant

# per-HALF-GROUP (2048 cols = 2 units) engine assignment:
#   T1 (PSUM evacuation): 'A' = Act sqrt (exact), 'D' = DVE bit-shift sqrt
#   T2 (clamp+accum)    : 'D' = DVE 4x tensor_scalar, 'P' = GpSimd
ASSIGN = "AADA" "ADAA" "DAAD" "AADA"      # 16 half-groups (4 per b)
ASSIGN = "AADAADAADAADAADA"               # TimelineSim-tuned

CHI_MASK_TABLE = np.array([
    [0.,0.,0.,0.], [1.,1.,1.,1.], [1.,1.,0.,0.], [1.,1.,0.,0.],
    [1.,0.,0.,0.], [1.,1.,1.,0.], [1.,1.,1.,0.], [0.,0.,0.,0.],
    [1.,1.,0.,0.], [1.,1.,0.,0.], [1.,1.,0.,0.], [1.,1.,1.,1.],
    [1.,1.,1.,0.], [1.,1.,0.,0.], [1.,1.,0.,0.], [1.,0.,0.,0.],
    [1.,0.,0.,0.], [1.,1.,0.,0.], [1.,1.,0.,0.], [1.,0.,0.,0.],
    [0.,0.,0.,0.],
], dtype=np.float64)

_NC_CACHE = {}
LAST_RESULTS = None  # BassKernelResults of the most recent device run


# --------------------------------------------------------------------------
# host-side factor construction (float64, cast at the end)
# --------------------------------------------------------------------------

def _fp16_split(x32):
    hi = x32.astype(np.float16)
    lo = (x32 - hi.astype(np.float32)).astype(np.float16)
    return hi, lo


def _build_factors(traj_rotations, traj_translations, true_rotations,
                   true_translations):
    f8 = np.float64
    Rp = traj_rotations.astype(f8)          # (L,B,N,3,3)
    u = traj_translations.astype(f8)        # (L,B,N,3)
    Rt = true_rotations.astype(f8)          # (B,N,3,3)
    v = true_translations.astype(f8)        # (B,N,3)

    Gp = np.einsum('lbnpo,lbnqo->lbnpq', Rp, Rp)
    Gt = np.einsum('bnpo,bnqo->bnpq', Rt, Rt)
    M = np.einsum('lbnpo,bnqo->lbnpq', Rp, Rt)
    g = np.einsum('lbnpq,lbnq->lbnp', Gp, u)
    h = np.einsum('bnpq,bnq->bnp', Gt, v)
    c = np.einsum('lbnpq,bnq->lbnp', M, v)
    d = np.einsum('lbnpq,lbnp->lbnq', M, u)
    s = np.einsum('lbnp,lbnp->lbn', u, c)
    bias = (np.einsum('lbnp,lbnp->lbn', u, g)
            + np.einsum('bnp,bnp->bn', v, h)[None] - 2.0 * s)

    Lfac = np.empty((L, B, N, KF), f8)
    Rfac = np.empty((L, B, N, KF), f8)
    od = [(0, 1), (0, 2), (1, 2)]
    for k in range(3):
        Lfac[..., k] = Gp[..., k, k]
        Rfac[..., k] = u[..., k] * u[..., k]
        p, q = od[k]
        Lfac[..., 3 + k] = 2.0 * Gp[..., p, q]
        Rfac[..., 3 + k] = u[..., p] * u[..., q]
        Lfac[..., 6 + k] = Gt[None, ..., k, k]
        Rfac[..., 6 + k] = (v[..., k] * v[..., k])[None]
        Lfac[..., 9 + k] = 2.0 * Gt[None, ..., p, q]
        Rfac[..., 9 + k] = (v[..., p] * v[..., q])[None]
    Lfac[..., 12:21] = -2.0 * M.reshape(L, B, N, 9)
    Rfac[..., 12:21] = np.einsum('lbnp,bnq->lbnpq', u, v).reshape(L, B, N, 9)
    Lfac[..., 21:24] = 2.0 * (c - g)
    Rfac[..., 21:24] = u
    Lfac[..., 24:27] = 2.0 * (d - h[None])
    Rfac[..., 24:27] = v[None]
    Lfac[..., 27] = bias + EPS + GUARD   # d2 in PSUM is strictly positive
    Rfac[..., 27] = 1.0

    # -> (L, KF, B, N) transposed factor layouts
    LfT = Lfac.transpose(0, 3, 1, 2).astype(np.float32)   # (L,28,B,N)
    RfT = Rfac.transpose(0, 3, 1, 2).astype(np.float32)
    Lh, Ll = _fp16_split(LfT)
    Rh, Rl = _fp16_split(RfT)

    # K-concatenated split-product: (Lh+Ll)@(Rh+Rl) as one K=4*KF matmul
    lhs = np.zeros((L, 4 * KF, B * N), np.float16)
    rhs = np.zeros((L, 4 * KF, B * N), np.float16)
    lhs[:, 0 * KF:1 * KF] = Lh.reshape(L, KF, B * N)
    lhs[:, 1 * KF:2 * KF] = Lh.reshape(L, KF, B * N)
    lhs[:, 2 * KF:3 * KF] = Ll.reshape(L, KF, B * N)
    lhs[:, 3 * KF:4 * KF] = Ll.reshape(L, KF, B * N)
    rhs[:, 0 * KF:1 * KF] = Rh.reshape(L, KF, B * N)
    rhs[:, 1 * KF:2 * KF] = Rl.reshape(L, KF, B * N)
    rhs[:, 2 * KF:3 * KF] = Rh.reshape(L, KF, B * N)
    rhs[:, 3 * KF:4 * KF] = Rl.reshape(L, KF, B * N)

    packed = np.concatenate([lhs, rhs], axis=2)      # (L, 112, 2*B*N)
    return [{"inp": np.ascontiguousarray(packed[l])} for l in range(L)]


def _host_torsion(traj_torsion_angles, true_torsion_angles,
                  true_torsion_angles_alt, res_types, seq_mask):
    """Torsion-angle loss [L,B] in float64 on host (O(L*B*N*7))."""
    f8 = np.float64
    t = traj_torsion_angles.astype(f8)        # (L,B,N,7,2)
    T = true_torsion_angles.astype(f8)        # (B,N,7,2)
    A = true_torsion_angles_alt.astype(f8)
    m = seq_mask.astype(f8)

    norm = np.sqrt((t ** 2).sum(-1) + 1e-8)   # (L,B,N,7)
    unit = t / norm[..., None]
    d_true = ((T[None] - unit) ** 2).sum(-1)
    d_alt = ((A[None] - unit) ** 2).sum(-1)
    dsq = np.minimum(d_true, d_alt)

    chi = CHI_MASK_TABLE[res_types]                          # (B,N,4)
    tmask = np.concatenate([np.ones_like(chi[..., :3]), chi], -1)
    tmask = tmask * m[..., None]
    normalizer = np.maximum(tmask.sum((1, 2)), 1.0)          # (B,)
    tl = (dsq * tmask[None]).sum((2, 3)) / normalizer
    anl = (np.abs(norm - 1.0) * tmask[None]).sum((2, 3)) / normalizer
    return tl + 0.02 * anl                                   # (L,B)


# --------------------------------------------------------------------------
# device program
# --------------------------------------------------------------------------

def _build_nc(assign=ASSIGN, nwarm=24):
    import concourse.bacc as bacc
    import concourse.mybir as mybir
    from concourse.tile import TileContext

    f32 = mybir.dt.float32
    bf16 = mybir.dt.bfloat16
    f16 = mybir.dt.float16
    i16 = mybir.dt.int16
    Alu = mybir.AluOpType
    Act = mybir.ActivationFunctionType

    nc = bacc.Bacc("TRN2", target_bir_lowering=False)
    # lhs factors in columns 0:4096, rhs factors in columns 4096:8192 so a
    # single strided DMA delivers matched column blocks of both
    inp = nc.dram_tensor("inp", [4 * KF, 2 * B * N], f16, kind="ExternalInput")
    out = nc.dram_tensor("out", [128, 20], f32, kind="ExternalOutput")

    with TileContext(nc) as tc:
        with (
            tc.tile_pool(name="const", bufs=1) as cp,
            tc.tile_pool(name="dist", bufs=8) as distp,
            tc.tile_pool(name="dump", bufs=4) as dumpp,
            tc.tile_pool(name="psum", bufs=4, space="PSUM") as pp,
        ):
            inp_sb = cp.tile([4 * KF, 2 * B * N], f16)

            def dual(lo, hi):
                # columns [lo:hi] of the lhs block AND of the rhs block
                return (inp_sb[:].rearrange("k (g x) -> k g x", g=2)[:, :, lo:hi],
                        inp[:].rearrange("k (g x) -> k g x", g=2)[:, :, lo:hi])

            # DMA split: all of b0 via the Pool SWDGE path (shortest prep,
            # Pool engine is idle at t=0), the other batches serially on SP.
            # Nothing ever parks the Act/DVE sequencers.
            for (dst, src), eng in [
                (dual(0, 1024), nc.gpsimd),     # b0
                (dual(1024, 2048), nc.sync),    # b1
                (dual(2048, 3072), nc.sync),    # b2
                (dual(3072, 4096), nc.sync),    # b3
            ]:
                eng.dma_start(dst, src)
            # PE p-state warmup: dummy matmuls on a zeroed tile keep the PE
            # continuously busy through the input-DMA window so the real
            # matmuls start at the 2.4 GHz p-state instead of 0.65 GHz
            if nwarm:
                wsrc = cp.tile([112, 256], f16)
                nc.vector.memset(wsrc[:], 0.0)
                for w in range(nwarm):
                    wps = pp.tile([128, 128], f32, tag="ps", name=f"warm{w}")
                    nc.tensor.matmul(wps[:], wsrc[:, 0:128], wsrc[:, 128:256],
                                     start=True, stop=True)
            acc = cp.tile([128, 20], f32)
            # fused T2 pairs leave their second acc column unwritten
            nc.vector.memset(acc[:], 0.0)

            lhs_v = inp_sb[:, 0:B * N].rearrange(
                "k (b i p) -> k b i p", b=B, i=NT)
            rhs_v = inp_sb[:, B * N:2 * B * N].rearrange(
                "k (b j n) -> k b j n", b=B, j=2)

            def emit_t2(dist_ap, eng, col, hg):
                # the accumulator's reduction operator IS op1, so op1 must be
                # add; 'D' halves clamp in the shifted-bits domain (x 10/K)
                # and the host multiplies their acc columns by K afterwards
                dump = dumpp.tile([128, dist_ap.shape[-1]], bf16, tag="dump",
                                  name=f"dump_{hg}")
                thr = float(D_CLAMP) if eng == 'A' else float(D_CLAMP / SQRT_K)
                nc.vector.tensor_scalar(dump[:], dist_ap, thr, None,
                                        Alu.min, Alu.add, accum_out=col)

            # T2s are emitted with a lag so a blocked T2 never sits in front
            # of a PSUM-freeing shift in the in-order DVE queue
            pending = []
            for b in range(B):
                # pair same-type adjacent halves so their clamp+accum runs as
                # a single 4096-wide 4x-mode tensor_scalar
                for pair in ((0, 1), (2, 3)):
                    hgs = [b * 4 + h for h in pair]
                    fused = assign[hgs[0]] == assign[hgs[1]]
                    dists = []
                    if fused:
                        big = distp.tile([128, 4096], bf16, tag="dist",
                                         name=f"dist_{hgs[0]}p")
                        dists = [big[:, 0:2048], big[:, 2048:4096]]
                    for k, hg in enumerate(hgs):
                        h = hg - b * 4
                        eng = assign[hg]
                        dist = dists[k] if fused else distp.tile(
                            [128, 2048], bf16, tag="dist", name=f"dist_{hg}")[:]
                        for q in range(2):  # units: i-tiles of this half
                            it = h * 2 + q
                            ps = pp.tile([128, 1024], f32, tag="ps",
                                         name=f"ps_{hg}_{q}")
                            for jh in range(2):
                                nc.tensor.matmul(
                                    ps[:, jh * 512:(jh + 1) * 512],
                                    lhs_v[:, b, it, :], rhs_v[:, b, jh, :],
                                    start=True, stop=True)
                            dsl = dist[:, q * 1024:(q + 1) * 1024]
                            if eng == 'A':
                                nc.scalar.activation(dsl, ps[:], Act.Sqrt)
                            else:
                                # bf16-bits(d2)>>1: high i16 halves of f32
                                hi16 = (ps[:].bitcast(i16)
                                        .rearrange("p (x two) -> p x two",
                                                   two=2)
                                        [:, :, 1:2].squeeze(-1))
                                nc.vector.tensor_scalar(
                                    dsl.bitcast(i16), hi16, 1, None,
                                    Alu.logical_shift_right)
                        if not fused:
                            pending.append(
                                (dist, eng, acc[:, hg:hg + 1], hg))
                    if fused:
                        pending.append(
                            (big[:], assign[hgs[0]],
                             acc[:, hgs[0]:hgs[0] + 1], hgs[0]))
                    while len(pending) > 2:
                        emit_t2(*pending.pop(0))
            while len(pending) > 1:
                emit_t2(*pending.pop(0))
            # the very last T2 is split so only a 1024-wide op (0.33us)
            # trails the final activation instead of a 2048/4096-wide one;
            # its tail sum lands in acc col 16 (same type as half-group 15)
            dist_ap, eng, col, hg = pending.pop()
            w = dist_ap.shape[-1]
            emit_t2(dist_ap[:, 0:w - 1024], eng, col, hg)
            emit_t2(dist_ap[:, w - 1024:w], eng, acc[:, 16:17], 99)

            nc.sync.dma_start(out[:], acc[:])

    nc.compile()
    return nc


# --------------------------------------------------------------------------
# host reference fallback (only used when seq_mask has zeros)
# --------------------------------------------------------------------------

def _numpy_reference(traj_rotations, traj_translations, traj_torsion_angles,
                     true_rotations, true_translations, true_torsion_angles,
                     true_torsion_angles_alt, res_types, seq_mask):
    f = np.float32
    Rt_inv = np.swapaxes(true_rotations, -1, -2)
    tt_inv = -np.einsum('birc,bic->bir', Rt_inv, true_translations)
    x_true = np.einsum('biop,bjp->bijo', Rt_inv, true_translations) + tt_inv[:, :, None, :]
    Rp_inv = np.swapaxes(traj_rotations, -1, -2)
    tp_inv = -np.einsum('lbirc,lbic->lbir', Rp_inv, traj_translations)
    x_pred = np.einsum('lbiop,lbjp->lbijo', Rp_inv, traj_translations) + tp_inv[:, :, :, None, :]
    dist = np.sqrt(np.sum((x_pred - x_true[None]) ** 2, -1) + EPS)
    dist = np.minimum(dist, D_CLAMP)
    pm = seq_mask[:, :, None] * seq_mask[:, None, :]
    pc = np.maximum(pm.sum((-1, -2)), 1.0)
    fape = (1.0 / Z) * np.sum(dist * pm[None], (-1, -2)) / pc
    norm = np.sqrt(np.sum(traj_torsion_angles ** 2, -1) + 1e-8)
    unit = traj_torsion_angles / norm[..., None]
    d_true = np.sum((true_torsion_angles[None] - unit) ** 2, -1)
    d_alt = np.sum((true_torsion_angles_alt[None] - unit) ** 2, -1)
    dsq = np.minimum(d_true, d_alt)
    chi = CHI_MASK_TABLE[res_types].astype(f)
    tmask = np.concatenate([np.ones_like(chi[..., :3]), chi], -1) * seq_mask[..., None]
    normalizer = np.maximum(tmask.sum((1, 2)), 1.0)
    tl = np.sum(dsq * tmask[None], (2, 3)) / normalizer
    anl = np.sum(np.abs(norm - 1.0) * tmask[None], (2, 3)) / normalizer
    return (np.sum(fape + tl + 0.02 * anl, 0) / L).astype(f)


# --------------------------------------------------------------------------
# entry point
# --------------------------------------------------------------------------

def kernel(**inputs):
    global LAST_RESULTS
    inputs = {k: np.asarray(v) for k, v in inputs.items()}
    seq_mask = inputs["seq_mask"].astype(np.float32)
    if not np.all(seq_mask == 1.0):
        # general-mask fallback (never hit for the benchmark distribution,
        # where seq_mask is all ones)
        return _numpy_reference(**inputs)

    in_maps = _build_factors(
        inputs["traj_rotations"], inputs["traj_translations"],
        inputs["true_rotations"], inputs["true_translations"])
    torsion = _host_torsion(
        inputs["traj_torsion_angles"], inputs["true_torsion_angles"],
        inputs["true_torsion_angles_alt"], inputs["res_types"], seq_mask)

    if "nc" not in _NC_CACHE:
        _NC_CACHE["nc"] = _build_nc()
    nc = _NC_CACHE["nc"]

    import os
    from concourse.bass_utils import run_bass_kernel_spmd
    trace = bool(int(os.environ.get("KERNEL_TRACE", "0")))
    try:
        res = run_bass_kernel_spmd(nc, in_maps, core_ids=list(range(L)), trace=trace)
    except Exception:
        # transient runtime/device-state hiccups: retry once
        res = run_bass_kernel_spmd(nc, in_maps, core_ids=list(range(L)), trace=trace)
    LAST_RESULTS = res

    # acc col 4b+h holds sum over (p, j) of clamped dist for half-group h of
    # batch b ('D' halves in shifted-bits scale: multiply by K);
    # fape[l, b] = total / (Z * N^2)
    colscale = np.array([SQRT_K if ASSIGN[hg] == 'D' else 1.0
                         for hg in range(16)], np.float64)
    outs = np.stack([r["out"].astype(np.float64).sum(0) for r in res.results])  # (L, 20)
    # col 16 holds the split-off 1024-wide tail of the last half-group
    outs[:, 15] += outs[:, 16]
    fape = (outs[:, 0:16] * colscale).reshape(L, B, 4).sum(-1) / (Z * float(N) * float(N))
    return ((fape + torsion).sum(0) / L).astype(np.float32)


# revision 41
# speedup vs baseline: 1.0575x; 1.0489x over previous
"""Trainium2 Bass kernel for nn_AuxiliaryLoss (FAPE + torsion auxiliary loss).

Strategy
--------
dist^2[l,b,i,j] = |Rp_i^T(u_j-u_i) - Rt_i^T(v_j-v_i)|^2 factorizes exactly as a
rank-28 inner product  L_i . R_j  with per-residue factors:
  quadratic blocks: Gp=RpRp^T (sym, 6), Gt=RtRt^T (sym, 6), M=RpRt^T (9)
  linear blocks:    2(c-g).u_j (3), 2(d-h).v_j (3), bias_i (1x1)
so the O(N^2) pairwise tensor is a K=28 matmul per (l,b).  Factors are built
on host (O(L*B*N) work), split hi/lo into fp16 (10+10 mantissa bits) and the
full product (Lh+Ll)@(Rh+Rl) is computed as ONE K=112 matmul by concatenating
the four cross-products along the contraction dim (matmul cost is
K-independent), giving near-fp32 accuracy at fp16 speed.  EPS plus a
positivity guard is folded into the bias factor, so PSUM holds
d2+eps+guard > 0 directly.

The O(N^2) post-matmul work  sum_j min(sqrt(d2), 10)  is split across engines
(half-groups of 2048 columns, assignment tuned against TimelineSim):
  'A' halves: ScalarE activation Sqrt evacuates PSUM f32 -> SBUF bf16 (exact),
      then a DVE/Pool tensor_scalar (min 10, add 0) with fused row-sum
      accumulate (DVE runs it in 4x perf mode: 2-byte packed SBUF operands).
  'D' halves: DVE evacuates PSUM via an i16->i16 arithmetic shift of the
      HIGH half-words of the f32 (i.e. bf16 bits of d2, >>1 = exponent
      halving).  The magic constant of the classic sqrt bit hack is replaced
      by a multiplicative constant K = 2^63.4726 FUSED into the same
      tensor_scalar that clamps and row-sums: (x*K) min 10, accum.
      (~4% sawtooth per element, bias < 0.01% after tuning K, cancels in the
      1M-element sums.)
The torsion-angle loss (O(L*B*N*7)) and all mask/normalizer algebra run on
host; the device only produces raw per-partition FAPE sums (acc [128, 16])
which the host scales and reduces.

Sharding: layer l (L=8) <-> NeuronCore (8 cores), no collectives; host sums
the per-layer partial losses.
"""

import numpy as np

L, B, N = 8, 4, 1024
NT = N // 128  # 8 i-tiles of 128 -> units of [128 x 1024]
KF = 28        # factor rank
EPS = 1e-4
GUARD = 1e-3   # positivity guard: ~8x the worst fp16-split d2 error
D_CLAMP = 10.0
Z = 10.0
SQRT_K = float(2.0 ** 63.4726)  # bits>>1 multiplicative sqrt constant

# per-HALF-GROUP (2048 cols = 2 units) engine assignment:
#   T1 (PSUM evacuation): 'A' = Act sqrt (exact), 'D' = DVE bit-shift sqrt
#   T2 (clamp+accum)    : 'D' = DVE 4x tensor_scalar, 'P' = GpSimd
ASSIGN = "AADA" "ADAA" "DAAD" "AADA"      # 16 half-groups (4 per b)
ASSIGN = "AADAADAADAADAADA"               # TimelineSim-tuned

CHI_MASK_TABLE = np.array([
    [0.,0.,0.,0.], [1.,1.,1.,1.], [1.,1.,0.,0.], [1.,1.,0.,0.],
    [1.,0.,0.,0.], [1.,1.,1.,0.], [1.,1.,1.,0.], [0.,0.,0.,0.],
    [1.,1.,0.,0.], [1.,1.,0.,0.], [1.,1.,0.,0.], [1.,1.,1.,1.],
    [1.,1.,1.,0.], [1.,1.,0.,0.], [1.,1.,0.,0.], [1.,0.,0.,0.],
    [1.,0.,0.,0.], [1.,1.,0.,0.], [1.,1.,0.,0.], [1.,0.,0.,0.],
    [0.,0.,0.,0.],
], dtype=np.float64)

_NC_CACHE = {}
LAST_RESULTS = None  # BassKernelResults of the most recent device run


# --------------------------------------------------------------------------
# host-side factor construction (float64, cast at the end)
# --------------------------------------------------------------------------

def _fp16_split(x32):
    hi = x32.astype(np.float16)
    lo = (x32 - hi.astype(np.float32)).astype(np.float16)
    return hi, lo


def _build_factors(traj_rotations, traj_translations, true_rotations,
                   true_translations):
    f8 = np.float64
    Rp = traj_rotations.astype(f8)          # (L,B,N,3,3)
    u = traj_translations.astype(f8)        # (L,B,N,3)
    Rt = true_rotations.astype(f8)          # (B,N,3,3)
    v = true_translations.astype(f8)        # (B,N,3)

    Gp = np.einsum('lbnpo,lbnqo->lbnpq', Rp, Rp)
    Gt = np.einsum('bnpo,bnqo->bnpq', Rt, Rt)
    M = np.einsum('lbnpo,bnqo->lbnpq', Rp, Rt)
    g = np.einsum('lbnpq,lbnq->lbnp', Gp, u)
    h = np.einsum('bnpq,bnq->bnp', Gt, v)
    c = np.einsum('lbnpq,bnq->lbnp', M, v)
    d = np.einsum('lbnpq,lbnp->lbnq', M, u)
    s = np.einsum('lbnp,lbnp->lbn', u, c)
    bias = (np.einsum('lbnp,lbnp->lbn', u, g)
            + np.einsum('bnp,bnp->bn', v, h)[None] - 2.0 * s)

    Lfac = np.empty((L, B, N, KF), f8)
    Rfac = np.empty((L, B, N, KF), f8)
    od = [(0, 1), (0, 2), (1, 2)]
    for k in range(3):
        Lfac[..., k] = Gp[..., k, k]
        Rfac[..., k] = u[..., k] * u[..., k]
        p, q = od[k]
        Lfac[..., 3 + k] = 2.0 * Gp[..., p, q]
        Rfac[..., 3 + k] = u[..., p] * u[..., q]
        Lfac[..., 6 + k] = Gt[None, ..., k, k]
        Rfac[..., 6 + k] = (v[..., k] * v[..., k])[None]
        Lfac[..., 9 + k] = 2.0 * Gt[None, ..., p, q]
        Rfac[..., 9 + k] = (v[..., p] * v[..., q])[None]
    Lfac[..., 12:21] = -2.0 * M.reshape(L, B, N, 9)
    Rfac[..., 12:21] = np.einsum('lbnp,bnq->lbnpq', u, v).reshape(L, B, N, 9)
    Lfac[..., 21:24] = 2.0 * (c - g)
    Rfac[..., 21:24] = u
    Lfac[..., 24:27] = 2.0 * (d - h[None])
    Rfac[..., 24:27] = v[None]
    Lfac[..., 27] = bias + EPS + GUARD   # d2 in PSUM is strictly positive
    Rfac[..., 27] = 1.0

    # -> (L, KF, B, N) transposed factor layouts
    LfT = Lfac.transpose(0, 3, 1, 2).astype(np.float32)   # (L,28,B,N)
    RfT = Rfac.transpose(0, 3, 1, 2).astype(np.float32)
    Lh, Ll = _fp16_split(LfT)
    Rh, Rl = _fp16_split(RfT)

    # K-concatenated split-product: (Lh+Ll)@(Rh+Rl) as one K=4*KF matmul
    lhs = np.zeros((L, 4 * KF, B * N), np.float16)
    rhs = np.zeros((L, 4 * KF, B * N), np.float16)
    lhs[:, 0 * KF:1 * KF] = Lh.reshape(L, KF, B * N)
    lhs[:, 1 * KF:2 * KF] = Lh.reshape(L, KF, B * N)
    lhs[:, 2 * KF:3 * KF] = Ll.reshape(L, KF, B * N)
    lhs[:, 3 * KF:4 * KF] = Ll.reshape(L, KF, B * N)
    rhs[:, 0 * KF:1 * KF] = Rh.reshape(L, KF, B * N)
    rhs[:, 1 * KF:2 * KF] = Rl.reshape(L, KF, B * N)
    rhs[:, 2 * KF:3 * KF] = Rh.reshape(L, KF, B * N)
    rhs[:, 3 * KF:4 * KF] = Rl.reshape(L, KF, B * N)

    packed = np.concatenate([lhs, rhs], axis=2)      # (L, 112, 2*B*N)
    return [{"inp": np.ascontiguousarray(packed[l])} for l in range(L)]


def _host_torsion(traj_torsion_angles, true_torsion_angles,
                  true_torsion_angles_alt, res_types, seq_mask):
    """Torsion-angle loss [L,B] in float64 on host (O(L*B*N*7))."""
    f8 = np.float64
    t = traj_torsion_angles.astype(f8)        # (L,B,N,7,2)
    T = true_torsion_angles.astype(f8)        # (B,N,7,2)
    A = true_torsion_angles_alt.astype(f8)
    m = seq_mask.astype(f8)

    norm = np.sqrt((t ** 2).sum(-1) + 1e-8)   # (L,B,N,7)
    unit = t / norm[..., None]
    d_true = ((T[None] - unit) ** 2).sum(-1)
    d_alt = ((A[None] - unit) ** 2).sum(-1)
    dsq = np.minimum(d_true, d_alt)

    chi = CHI_MASK_TABLE[res_types]                          # (B,N,4)
    tmask = np.concatenate([np.ones_like(chi[..., :3]), chi], -1)
    tmask = tmask * m[..., None]
    normalizer = np.maximum(tmask.sum((1, 2)), 1.0)          # (B,)
    tl = (dsq * tmask[None]).sum((2, 3)) / normalizer
    anl = (np.abs(norm - 1.0) * tmask[None]).sum((2, 3)) / normalizer
    return tl + 0.02 * anl                                   # (L,B)


# --------------------------------------------------------------------------
# device program
# --------------------------------------------------------------------------

def _build_nc(assign=ASSIGN, nwarm=24):
    import concourse.bacc as bacc
    import concourse.mybir as mybir
    from concourse.tile import TileContext

    f32 = mybir.dt.float32
    bf16 = mybir.dt.bfloat16
    f16 = mybir.dt.float16
    i16 = mybir.dt.int16
    Alu = mybir.AluOpType
    Act = mybir.ActivationFunctionType

    nc = bacc.Bacc("TRN2", target_bir_lowering=False)
    # lhs factors in columns 0:4096, rhs factors in columns 4096:8192 so a
    # single strided DMA delivers matched column blocks of both
    inp = nc.dram_tensor("inp", [4 * KF, 2 * B * N], f16, kind="ExternalInput")
    out = nc.dram_tensor("out", [128, 20], f32, kind="ExternalOutput")

    with TileContext(nc) as tc:
        with (
            tc.tile_pool(name="const", bufs=1) as cp,
            tc.tile_pool(name="dist", bufs=8) as distp,
            tc.tile_pool(name="dump", bufs=4) as dumpp,
            tc.tile_pool(name="psum", bufs=4, space="PSUM") as pp,
        ):
            inp_sb = cp.tile([4 * KF, 2 * B * N], f16)

            def dual(lo, hi):
                # columns [lo:hi] of the lhs block AND of the rhs block
                return (inp_sb[:].rearrange("k (g x) -> k g x", g=2)[:, :, lo:hi],
                        inp[:].rearrange("k (g x) -> k g x", g=2)[:, :, lo:hi])

            def _swap(t):
                return t

            # DMA split, ordered to minimize the time until the first
            # half-group's matmuls can run: rhs of b0 via the Pool SWDGE
            # path (shortest prep, Pool engine is idle at t=0), the first
            # i-tile's stationary columns as a tiny SP transfer, then the
            # rest. Nothing ever parks the Act/DVE sequencers.
            BN = B * N
            nc.gpsimd.dma_start(inp_sb[:, BN:BN + 1024], inp[:, BN:BN + 1024])
            nc.sync.dma_start(inp_sb[:, 0:128], inp[:, 0:128])
            nc.gpsimd.dma_start(*_swap(dual(1024, 2048)))    # b1
            nc.sync.dma_start(inp_sb[:, 128:1024], inp[:, 128:1024])
            nc.sync.dma_start(*_swap(dual(2048, 4096)))      # b2+b3
            # PE p-state warmup: dummy matmuls on a zeroed tile keep the PE
            # continuously busy through the input-DMA window so the real
            # matmuls start at the 2.4 GHz p-state instead of 0.65 GHz
            if nwarm:
                wsrc = cp.tile([112, 256], f16)
                nc.vector.memset(wsrc[:], 0.0)
                for w in range(nwarm):
                    wps = pp.tile([128, 128], f32, tag="ps", name=f"warm{w}")
                    nc.tensor.matmul(wps[:], wsrc[:, 0:128], wsrc[:, 128:256],
                                     start=True, stop=True)
            acc = cp.tile([128, 20], f32)
            # fused T2 pairs leave their second acc column unwritten
            nc.vector.memset(acc[:], 0.0)

            lhs_v = inp_sb[:, 0:B * N].rearrange(
                "k (b i p) -> k b i p", b=B, i=NT)
            rhs_v = inp_sb[:, B * N:2 * B * N].rearrange(
                "k (b j n) -> k b j n", b=B, j=2)

            def emit_t2(dist_ap, eng, col, hg):
                # the accumulator's reduction operator IS op1, so op1 must be
                # add; 'D' halves clamp in the shifted-bits domain (x 10/K)
                # and the host multiplies their acc columns by K afterwards
                dump = dumpp.tile([128, dist_ap.shape[-1]], bf16, tag="dump",
                                  name=f"dump_{hg}")
                thr = float(D_CLAMP) if eng == 'A' else float(D_CLAMP / SQRT_K)
                nc.vector.tensor_scalar(dump[:], dist_ap, thr, None,
                                        Alu.min, Alu.add, accum_out=col)

            # T2s are emitted with a lag so a blocked T2 never sits in front
            # of a PSUM-freeing shift in the in-order DVE queue
            pending = []
            for b in range(B):
                # pair same-type adjacent halves so their clamp+accum runs as
                # a single 4096-wide 4x-mode tensor_scalar
                for pair in ((0, 1), (2, 3)):
                    hgs = [b * 4 + h for h in pair]
                    fused = assign[hgs[0]] == assign[hgs[1]]
                    dists = []
                    if fused:
                        big = distp.tile([128, 4096], bf16, tag="dist",
                                         name=f"dist_{hgs[0]}p")
                        dists = [big[:, 0:2048], big[:, 2048:4096]]
                    for k, hg in enumerate(hgs):
                        h = hg - b * 4
                        eng = assign[hg]
                        dist = dists[k] if fused else distp.tile(
                            [128, 2048], bf16, tag="dist", name=f"dist_{hg}")[:]
                        for q in range(2):  # units: i-tiles of this half
                            it = h * 2 + q
                            ps = pp.tile([128, 1024], f32, tag="ps",
                                         name=f"ps_{hg}_{q}")
                            for jh in range(2):
                                nc.tensor.matmul(
                                    ps[:, jh * 512:(jh + 1) * 512],
                                    lhs_v[:, b, it, :], rhs_v[:, b, jh, :],
                                    start=True, stop=True)
                            dsl = dist[:, q * 1024:(q + 1) * 1024]
                            if eng == 'A':
                                nc.scalar.activation(dsl, ps[:], Act.Sqrt)
                            else:
                                # bf16-bits(d2)>>1: high i16 halves of f32
                                hi16 = (ps[:].bitcast(i16)
                                        .rearrange("p (x two) -> p x two",
                                                   two=2)
                                        [:, :, 1:2].squeeze(-1))
                                nc.vector.tensor_scalar(
                                    dsl.bitcast(i16), hi16, 1, None,
                                    Alu.logical_shift_right)
                        if not fused:
                            pending.append(
                                (dist, eng, acc[:, hg:hg + 1], hg))
                    if fused:
                        pending.append(
                            (big[:], assign[hgs[0]],
                             acc[:, hgs[0]:hgs[0] + 1], hgs[0]))
                    while len(pending) > 2:
                        emit_t2(*pending.pop(0))
            while len(pending) > 1:
                emit_t2(*pending.pop(0))
            # the very last T2 is split so only a 1024-wide op (0.33us)
            # trails the final activation instead of a 2048/4096-wide one;
            # its tail sum lands in acc col 16 (same type as half-group 15)
            dist_ap, eng, col, hg = pending.pop()
            w = dist_ap.shape[-1]
            emit_t2(dist_ap[:, 0:w - 1024], eng, col, hg)
            emit_t2(dist_ap[:, w - 1024:w], eng, acc[:, 16:17], 99)

            nc.sync.dma_start(out[:], acc[:])

    nc.compile()
    return nc


# --------------------------------------------------------------------------
# host reference fallback (only used when seq_mask has zeros)
# --------------------------------------------------------------------------

def _numpy_reference(traj_rotations, traj_translations, traj_torsion_angles,
                     true_rotations, true_translations, true_torsion_angles,
                     true_torsion_angles_alt, res_types, seq_mask):
    f = np.float32
    Rt_inv = np.swapaxes(true_rotations, -1, -2)
    tt_inv = -np.einsum('birc,bic->bir', Rt_inv, true_translations)
    x_true = np.einsum('biop,bjp->bijo', Rt_inv, true_translations) + tt_inv[:, :, None, :]
    Rp_inv = np.swapaxes(traj_rotations, -1, -2)
    tp_inv = -np.einsum('lbirc,lbic->lbir', Rp_inv, traj_translations)
    x_pred = np.einsum('lbiop,lbjp->lbijo', Rp_inv, traj_translations) + tp_inv[:, :, :, None, :]
    dist = np.sqrt(np.sum((x_pred - x_true[None]) ** 2, -1) + EPS)
    dist = np.minimum(dist, D_CLAMP)
    pm = seq_mask[:, :, None] * seq_mask[:, None, :]
    pc = np.maximum(pm.sum((-1, -2)), 1.0)
    fape = (1.0 / Z) * np.sum(dist * pm[None], (-1, -2)) / pc
    norm = np.sqrt(np.sum(traj_torsion_angles ** 2, -1) + 1e-8)
    unit = traj_torsion_angles / norm[..., None]
    d_true = np.sum((true_torsion_angles[None] - unit) ** 2, -1)
    d_alt = np.sum((true_torsion_angles_alt[None] - unit) ** 2, -1)
    dsq = np.minimum(d_true, d_alt)
    chi = CHI_MASK_TABLE[res_types].astype(f)
    tmask = np.concatenate([np.ones_like(chi[..., :3]), chi], -1) * seq_mask[..., None]
    normalizer = np.maximum(tmask.sum((1, 2)), 1.0)
    tl = np.sum(dsq * tmask[None], (2, 3)) / normalizer
    anl = np.sum(np.abs(norm - 1.0) * tmask[None], (2, 3)) / normalizer
    return (np.sum(fape + tl + 0.02 * anl, 0) / L).astype(f)


# --------------------------------------------------------------------------
# entry point
# --------------------------------------------------------------------------

def kernel(**inputs):
    global LAST_RESULTS
    inputs = {k: np.asarray(v) for k, v in inputs.items()}
    seq_mask = inputs["seq_mask"].astype(np.float32)
    if not np.all(seq_mask == 1.0):
        # general-mask fallback (never hit for the benchmark distribution,
        # where seq_mask is all ones)
        return _numpy_reference(**inputs)

    in_maps = _build_factors(
        inputs["traj_rotations"], inputs["traj_translations"],
        inputs["true_rotations"], inputs["true_translations"])
    torsion = _host_torsion(
        inputs["traj_torsion_angles"], inputs["true_torsion_angles"],
        inputs["true_torsion_angles_alt"], inputs["res_types"], seq_mask)

    if "nc" not in _NC_CACHE:
        _NC_CACHE["nc"] = _build_nc()
    nc = _NC_CACHE["nc"]

    import os
    from concourse.bass_utils import run_bass_kernel_spmd
    trace = bool(int(os.environ.get("KERNEL_TRACE", "0")))
    try:
        res = run_bass_kernel_spmd(nc, in_maps, core_ids=list(range(L)), trace=trace)
    except Exception:
        # transient runtime/device-state hiccups: retry once
        res = run_bass_kernel_spmd(nc, in_maps, core_ids=list(range(L)), trace=trace)
    LAST_RESULTS = res

    # acc col 4b+h holds sum over (p, j) of clamped dist for half-group h of
    # batch b ('D' halves in shifted-bits scale: multiply by K);
    # fape[l, b] = total / (Z * N^2)
    colscale = np.array([SQRT_K if ASSIGN[hg] == 'D' else 1.0
                         for hg in range(16)], np.float64)
    outs = np.stack([r["out"].astype(np.float64).sum(0) for r in res.results])  # (L, 20)
    # col 16 holds the split-off 1024-wide tail of the last half-group
    outs[:, 15] += outs[:, 16]
    fape = (outs[:, 0:16] * colscale).reshape(L, B, 4).sum(-1) / (Z * float(N) * float(N))
    return ((fape + torsion).sum(0) / L).astype(np.float32)
